# revision 1
# baseline (speedup 1.0000x reference)
"""Trainium2 Bass kernel for nn_AlternateConvolution (gnn_message_passing).

Computation (reference):
    w  = (H_e @ p.T)[:, 0]                    # [NE]
    M1 = (T * w) @ T.T                        # [NV, NV]  (symmetric)
    A  = (eye + (1-eye)*M1) * adj_v
    ret = A @ (H_v @ weight) + bias           # [NV, OUT_V]
    return (ret, H_e)

Distribution: output-row shard over 8 NeuronCores (256 rows each); each core
streams the full T^T in bf16 (no collectives). GEMM1 is computed in the
transposed orientation -- psum R[j, r] = M1[r, j] -- so the adj_v-masked
result is directly in lhsT layout for GEMM2 (no on-chip transposes).
"""

import os
import sys
import types

import numpy as np
import ml_dtypes

NV, NE = 2048, 8192
IN_V, OUT_V, IN_E = 128, 128, 128
NCORES = 8
R = NV // NCORES          # 256 output rows per core
JT = NV // 128            # 16 j-panels
KO = NE // 128            # 64 k-tiles

BF16 = ml_dtypes.bfloat16

_cache = {}
last_exec_time_ns = None
last_results = None


def _ensure_ntff_hook():
    """Register the NTFF profiling hook if the image's antenv lacks it."""
    try:
        import antenv  # noqa: F401
        import antenv.axon_hooks  # noqa: F401
        return
    except ImportError:
        pass
    try:
        import antenv
        from trn_agent_boot.trn_boot import _ntff_profile_via_ctypes

        hook = _ntff_profile_via_ctypes("/opt/axon/libaxon_pjrt.so")
        mod = types.ModuleType("antenv.axon_hooks")
        mod.get_axon_ntff_profile_hook = lambda: hook
        mod.set_axon_ntff_profile_hook = lambda h: None
        sys.modules["antenv.axon_hooks"] = mod
        antenv.axon_hooks = mod
    except Exception:
        pass


def _build():
    import concourse.mybir as mybir
    import concourse.tile as tile
    from concourse import bacc

    F32 = mybir.dt.float32
    B16 = mybir.dt.bfloat16

    nc = bacc.Bacc("TRN2", target_bir_lowering=False, debug=False,
                   num_devices=NCORES)

    TTp = nc.declare_dram_parameter("TTp", [JT, 128, KO, 128], B16, isOutput=False)
    slab = nc.declare_dram_parameter("slab", [128, KO, R], B16, isOutput=False)
    HeT = nc.declare_dram_parameter("HeT", [128, KO, 128], F32, isOutput=False)
    pT = nc.declare_dram_parameter("pT", [128, 1], F32, isOutput=False)
    adjT = nc.declare_dram_parameter("adjT", [128, JT, R], F32, isOutput=False)
    HvT = nc.declare_dram_parameter("HvT", [128, JT, 128], F32, isOutput=False)
    HvTr = nc.declare_dram_parameter("HvTr", [128, R], F32, isOutput=False)
    Wp = nc.declare_dram_parameter("W", [128, 128], F32, isOutput=False)
    dvals = nc.declare_dram_parameter("dvals", [128, 2], F32, isOutput=False)
    biasb = nc.declare_dram_parameter("biasb", [128, 128], F32, isOutput=False)
    out = nc.declare_dram_parameter("out", [R, OUT_V], F32, isOutput=True)

    with tile.TileContext(nc) as tc:
        with (
            tc.tile_pool(name="pers", bufs=1) as pers,
            tc.tile_pool(name="panels", bufs=3) as panels,
            tc.tile_pool(name="amtp", bufs=3) as amtp,
            tc.tile_pool(name="ps_g", bufs=2, space="PSUM") as ps_g,
            tc.tile_pool(name="ps_mm", bufs=4, space="PSUM") as ps_mm,
            tc.tile_pool(name="ps_ret", bufs=2, space="PSUM") as ps_ret,
        ):
            # ---- persistent SBUF tiles + input DMAs ----
            HeT_sb = pers.tile([128, KO, 128], F32)
            nc.sync.dma_start(HeT_sb[:], HeT[:])
            pT_sb = pers.tile([128, 1], F32)
            nc.sync.dma_start(pT_sb[:], pT[:])
            slab_sb = pers.tile([128, KO, R], B16)
            nc.sync.dma_start(slab_sb[:], slab[:])
            adjT_sb = pers.tile([128, JT, R], F32)
            nc.sync.dma_start(adjT_sb[:], adjT[:])
            HvT_sb = pers.tile([128, JT, 128], F32)
            nc.sync.dma_start(HvT_sb[:], HvT[:])
            HvTr_sb = pers.tile([128, R], F32)
            nc.sync.dma_start(HvTr_sb[:], HvTr[:])
            W_sb = pers.tile([128, 128], F32)
            nc.sync.dma_start(W_sb[:], Wp[:])
            dvals_sb = pers.tile([128, 2], F32)
            nc.sync.dma_start(dvals_sb[:], dvals[:])
            biasb_sb = pers.tile([128, 128], F32)
            nc.sync.dma_start(biasb_sb[:], biasb[:])

            w_sb = pers.tile([128, KO], F32)
            G_sb = pers.tile([128, JT, 128], B16)
            Grows_sb = pers.tile([128, 2, 128], F32)
            scaled_sb = pers.tile([128, KO, R], B16)

            # ---- w = H_e @ p^T  (fp32, exact) ----
            w_ps = ps_g.tile([128, 128], mybir.dt.float32, tag="gps")
            for t in range(KO):
                nc.tensor.matmul(w_ps[:, t : t + 1], lhsT=HeT_sb[:, t, :],
                                 rhs=pT_sb[:], start=True, stop=True)
            nc.vector.tensor_copy(w_sb[:], w_ps[:, :KO])

            # ---- G = H_v @ weight  (fp32 -> bf16) ----
            for jt in range(JT):
                g_ps = ps_g.tile([128, 128], mybir.dt.float32, tag="gps")
                nc.tensor.matmul(g_ps[:], lhsT=HvT_sb[:, jt, :], rhs=W_sb[:],
                                 start=True, stop=True)
                nc.vector.tensor_copy(G_sb[:, jt, :], g_ps[:])
            # rows of G owned by this core (for the diagonal term), fp32
            for rh in range(2):
                gr_ps = ps_g.tile([128, 128], mybir.dt.float32, tag="gps")
                nc.tensor.matmul(gr_ps[:], lhsT=HvTr_sb[:, rh * 128 : (rh + 1) * 128],
                                 rhs=W_sb[:], start=True, stop=True)
                nc.vector.tensor_copy(Grows_sb[:, rh, :], gr_ps[:])

            # ---- scaled slab: (T^T[:, rows] * w[k]) in bf16 ----
            for t in range(KO):
                nc.vector.tensor_scalar_mul(scaled_sb[:, t, :], slab_sb[:, t, :],
                                            w_sb[:, t : t + 1])

            # ---- main loop: R[j, r] = sum_k TT[k,j] * w[k]*TT[k,r] ----
            retps = [ps_ret.tile([128, 128], mybir.dt.float32, tag="ret",
                                 name=f"retps{rh}") for rh in range(2)]
            for jt in range(JT):
                panel = panels.tile([128, KO, 128], B16, tag="panel")
                nc.sync.dma_start(panel[:], TTp[jt])
                ps = ps_mm.tile([128, R], mybir.dt.float32, tag="mm")
                for ko in range(KO):
                    nc.tensor.matmul(ps[:], lhsT=panel[:, ko, :],
                                     rhs=scaled_sb[:, ko, :],
                                     start=(ko == 0), stop=(ko == KO - 1))
                # mask with adj_v (diag pre-zeroed on host), cast to bf16
                amt = amtp.tile([128, R], B16, tag="amt")
                nc.vector.tensor_tensor(amt[:], ps[:], adjT_sb[:, jt, :],
                                        mybir.AluOpType.mult)
                # GEMM2 incremental accumulation: ret[r, f] += amt^T @ G[jt]
                for rh in range(2):
                    nc.tensor.matmul(retps[rh][:],
                                     lhsT=amt[:, rh * 128 : (rh + 1) * 128],
                                     rhs=G_sb[:, jt, :],
                                     start=(jt == 0), stop=(jt == JT - 1))

            # ---- epilogue: + diag(adj_v)*G_rows + bias ----
            for rh in range(2):
                tmp = pers.tile([128, 128], F32, name=f"tmp{rh}")
                nc.vector.tensor_scalar_mul(tmp[:], Grows_sb[:, rh, :],
                                            dvals_sb[:, rh : rh + 1])
                nc.vector.tensor_add(tmp[:], tmp[:], retps[rh][:])
                nc.vector.tensor_add(tmp[:], tmp[:], biasb_sb[:])
                nc.sync.dma_start(out[rh * 128 : (rh + 1) * 128, :], tmp[:])

    nc.finalize()
    return nc


def kernel(H_v, H_e, adj_e, adj_v, T, weight, p, bias):
    global last_exec_time_ns, last_results
    _ensure_ntff_hook()
    from concourse.bass_utils import run_bass_kernel_spmd

    H_v = np.asarray(H_v, np.float32)
    H_e = np.asarray(H_e, np.float32)
    adj_v = np.asarray(adj_v, np.float32)
    T = np.asarray(T, np.float32)
    weight = np.asarray(weight, np.float32)
    p = np.asarray(p, np.float32).reshape(1, IN_E)
    bias = np.asarray(bias, np.float32).reshape(OUT_V)

    if "nc" not in _cache:
        _cache["nc"] = _build()
    nc = _cache["nc"]

    # ---- host-side sharding / layout prep (no reference math here) ----
    TT = np.ascontiguousarray(T.T)                                   # [NE, NV]
    TTb = TT.astype(BF16)
    # TTp[jt, p, ko, j] = TT[ko*128+p, jt*128+j]; per-panel contiguous
    TTp = np.ascontiguousarray(
        TTb.reshape(KO, 128, JT, 128).transpose(2, 1, 0, 3))
    HeT = np.ascontiguousarray(
        H_e.reshape(KO, 128, IN_E).transpose(2, 0, 1))               # [128, KO, 128]
    pT = np.ascontiguousarray(p.T)                                   # [128, 1]
    A0 = adj_v.copy()
    np.fill_diagonal(A0, 0.0)
    diag = np.ascontiguousarray(np.diag(adj_v))                      # [NV]
    HvT = np.ascontiguousarray(H_v.T)                                # [128, NV]
    HvT3 = HvT.reshape(IN_V, JT, 128)
    bias_b = np.ascontiguousarray(
        np.broadcast_to(bias.reshape(1, OUT_V), (128, OUT_V)))

    in_maps = []
    for c in range(NCORES):
        rows = slice(c * R, (c + 1) * R)
        slab_c = np.ascontiguousarray(
            TTb[:, rows].reshape(KO, 128, R).transpose(1, 0, 2))     # [128, KO, R]
        adjT_c = np.ascontiguousarray(
            A0[rows, :].T.reshape(JT, 128, R).transpose(1, 0, 2))    # [128, JT, R]
        dvals_c = np.ascontiguousarray(diag[rows].reshape(2, 128).T) # [128, 2]
        HvTr_c = np.ascontiguousarray(HvT[:, rows])                  # [128, R]
        in_maps.append({
            "TTp": TTp, "slab": slab_c, "HeT": HeT, "pT": pT,
            "adjT": adjT_c, "HvT": HvT3, "HvTr": HvTr_c, "W": weight,
            "dvals": dvals_c, "biasb": bias_b,
        })

    trace = bool(int(os.environ.get("KERNEL_TRACE", "0")))
    res = run_bass_kernel_spmd(nc, in_maps, list(range(NCORES)), trace=trace)
    last_exec_time_ns = res.exec_time_ns
    last_results = res

    ret = np.concatenate([res.results[c]["out"] for c in range(NCORES)], axis=0)
    return (ret, H_e)


# revision 3
# speedup vs baseline: 1.3540x; 1.3540x over previous
"""Trainium2 Bass kernel for nn_AlternateConvolution (gnn_message_passing).

Computation (reference):
    w  = (H_e @ p.T)[:, 0]                    # [NE]
    M1 = (T * w) @ T.T                        # [NV, NV]  (symmetric)
    A  = (eye + (1-eye)*M1) * adj_v
    ret = A @ (H_v @ weight) + bias           # [NV, OUT_V]
    return (ret, H_e)

Distribution: output-row shard over 8 NeuronCores (256 rows each); each core
streams the full T^T in bf16 (no collectives). The big GEMM keeps the
scaled 256-row slab as the stationary operand and streams T^T k-tiles
through the moving port (N=512), so LDWEIGHTS traffic stays tiny.
"""

import os
import sys
import types

import numpy as np
import ml_dtypes

NV, NE = 2048, 8192
IN_V, OUT_V, IN_E = 128, 128, 128
NCORES = 8
R = NV // NCORES          # 256 output rows per core
JT = NV // 128            # 16 j-tiles
KO = NE // 128            # 64 k-tiles
HCH = 8                   # chunks for head-critical DMAs

BF16 = ml_dtypes.bfloat16

_cache = {}
last_exec_time_ns = None
last_results = None


def _ensure_ntff_hook():
    """Register the NTFF profiling hook if the image's antenv lacks it."""
    try:
        import antenv  # noqa: F401
        import antenv.axon_hooks  # noqa: F401
        return
    except ImportError:
        pass
    try:
        import antenv
        from trn_agent_boot.trn_boot import _ntff_profile_via_ctypes

        hook = _ntff_profile_via_ctypes("/opt/axon/libaxon_pjrt.so")
        mod = types.ModuleType("antenv.axon_hooks")
        mod.get_axon_ntff_profile_hook = lambda: hook
        mod.set_axon_ntff_profile_hook = lambda h: None
        sys.modules["antenv.axon_hooks"] = mod
        antenv.axon_hooks = mod
    except Exception:
        pass


def _build():
    import concourse.mybir as mybir
    import concourse.tile as tile
    from concourse import bacc

    F32 = mybir.dt.float32
    B16 = mybir.dt.bfloat16

    nc = bacc.Bacc("TRN2", target_bir_lowering=False, debug=False,
                   num_devices=NCORES)

    TTr = nc.declare_dram_parameter("TTr", [KO, 128, NV], B16, isOutput=False)
    slab = nc.declare_dram_parameter("slab", [128, KO, R], B16, isOutput=False)
    HeT = nc.declare_dram_parameter("HeT", [128, KO, 128], B16, isOutput=False)
    pT = nc.declare_dram_parameter("pT", [128, 1], B16, isOutput=False)
    adjR = nc.declare_dram_parameter("adjR", [128, 2, NV], F32, isOutput=False)
    HvT = nc.declare_dram_parameter("HvT", [128, JT, 128], B16, isOutput=False)
    HvTr = nc.declare_dram_parameter("HvTr", [128, R], B16, isOutput=False)
    Wp = nc.declare_dram_parameter("W", [128, 128], B16, isOutput=False)
    dvals = nc.declare_dram_parameter("dvals", [128, 2], F32, isOutput=False)
    biasb = nc.declare_dram_parameter("biasb", [128, 128], F32, isOutput=False)
    eye = nc.declare_dram_parameter("eye", [128, 128], B16, isOutput=False)
    out = nc.declare_dram_parameter("out", [R, OUT_V], F32, isOutput=True)

    with tile.TileContext(nc) as tc:
        with (
            tc.tile_pool(name="pers", bufs=1) as pers,
            tc.tile_pool(name="ktp", bufs=6) as ktp,
            tc.tile_pool(name="amtp", bufs=3) as amtp,
            tc.tile_pool(name="PS", bufs=8, space="PSUM") as PS,
        ):
            # ---- persistent SBUF tiles + input DMAs ----
            # head-critical first, chunked so the first w matmul can start early
            HeT_sb = pers.tile([128, KO, 128], B16)
            for i in range(HCH):
                s = slice(i * (KO // HCH), (i + 1) * (KO // HCH))
                nc.sync.dma_start(HeT_sb[:, s, :], HeT[:, s, :])
            pT_sb = pers.tile([128, 1], B16)
            nc.sync.dma_start(pT_sb[:], pT[:])
            slab_sb = pers.tile([128, KO, R], B16)
            for i in range(HCH):
                s = slice(i * (KO // HCH), (i + 1) * (KO // HCH))
                nc.sync.dma_start(slab_sb[:, s, :], slab[:, s, :])
            HvT_sb = pers.tile([128, JT, 128], B16)
            nc.sync.dma_start(HvT_sb[:], HvT[:])
            HvTr_sb = pers.tile([128, R], B16)
            nc.sync.dma_start(HvTr_sb[:], HvTr[:])
            W_sb = pers.tile([128, 128], B16)
            nc.sync.dma_start(W_sb[:], Wp[:])
            dvals_sb = pers.tile([128, 2], F32)
            nc.sync.dma_start(dvals_sb[:], dvals[:])
            biasb_sb = pers.tile([128, 128], F32)
            nc.sync.dma_start(biasb_sb[:], biasb[:])
            eye_sb = pers.tile([128, 128], B16)
            nc.sync.dma_start(eye_sb[:], eye[:])
            adjR_sb = pers.tile([128, 2, NV], F32)
            nc.sync.dma_start(adjR_sb[:], adjR[:])

            w_sb = pers.tile([128, KO], F32)
            G_sb = pers.tile([128, JT, 128], B16)
            Grows_sb = pers.tile([128, 2, 128], F32)
            scaled_sb = pers.tile([128, KO, R], B16)

            # ---- w = H_e @ p^T ----
            w_ps = PS.tile([128, 512], mybir.dt.float32, tag="bank", name="w_ps")
            for t in range(KO):
                nc.tensor.matmul(w_ps[:, t : t + 1], lhsT=HeT_sb[:, t, :],
                                 rhs=pT_sb[:], start=True, stop=True)
            nc.vector.tensor_copy(w_sb[:], w_ps[:, :KO])

            # ---- G = H_v @ weight ----
            for jt in range(JT):
                g_ps = PS.tile([128, 512], mybir.dt.float32, tag="bank")
                nc.tensor.matmul(g_ps[:, :128], lhsT=HvT_sb[:, jt, :],
                                 rhs=W_sb[:], start=True, stop=True)
                nc.vector.tensor_copy(G_sb[:, jt, :], g_ps[:, :128])
            for rh in range(2):
                gr_ps = PS.tile([128, 512], mybir.dt.float32, tag="bank")
                nc.tensor.matmul(gr_ps[:, :128],
                                 lhsT=HvTr_sb[:, rh * 128 : (rh + 1) * 128],
                                 rhs=W_sb[:], start=True, stop=True)
                nc.vector.tensor_copy(Grows_sb[:, rh, :], gr_ps[:, :128])

            # ---- scaled slab: (T^T[:, rows] * w[k]) in bf16 ----
            for t in range(KO):
                nc.vector.tensor_scalar_mul(scaled_sb[:, t, :], slab_sb[:, t, :],
                                            w_sb[:, t : t + 1])

            # ---- GEMM1: psA[rh, jc] += scaled[:,ko,rh]^T @ TT[ko][:, jc] ----
            psA = [PS.tile([128, 512], mybir.dt.float32, tag="bank",
                           name=f"psA_{i}") for i in range(8)]
            for ko in range(KO):
                kt = ktp.tile([128, NV], B16, tag="kt")
                nc.sync.dma_start(kt[:], TTr[ko])
                for rh in range(2):
                    for jc in range(4):
                        nc.tensor.matmul(
                            psA[rh * 4 + jc][:],
                            lhsT=scaled_sb[:, ko, rh * 128 : (rh + 1) * 128],
                            rhs=kt[:, jc * 512 : (jc + 1) * 512],
                            start=(ko == 0), stop=(ko == KO - 1))

            # ---- mask with adj_v (diag pre-zeroed on host), cast to bf16 ----
            amk_sb = pers.tile([128, 2, NV], B16)
            for rh in range(2):
                for jc in range(4):
                    cs = slice(jc * 512, (jc + 1) * 512)
                    nc.vector.tensor_tensor(amk_sb[:, rh, cs], psA[rh * 4 + jc][:],
                                            adjR_sb[:, rh, cs],
                                            mybir.AluOpType.mult)

            # ---- transpose + GEMM2 (incremental accumulation) ----
            retps = [PS.tile([128, 512], mybir.dt.float32, tag="bank",
                             name=f"retps{rh}") for rh in range(2)]
            for jt in range(JT):
                tp = PS.tile([128, 512], B16, tag="bank")
                for rh in range(2):
                    nc.tensor.transpose(tp[:, rh * 128 : (rh + 1) * 128],
                                        amk_sb[:, rh, jt * 128 : (jt + 1) * 128],
                                        eye_sb[:])
                amt = amtp.tile([128, 256], B16, tag="amt")
                nc.vector.tensor_copy(amt[:], tp[:, :256])
                for rh in range(2):
                    nc.tensor.matmul(retps[rh][:, :128],
                                     lhsT=amt[:, rh * 128 : (rh + 1) * 128],
                                     rhs=G_sb[:, jt, :],
                                     start=(jt == 0), stop=(jt == JT - 1))

            # ---- epilogue: + diag(adj_v)*G_rows + bias ----
            for rh in range(2):
                tmp = pers.tile([128, 128], F32, name=f"tmp{rh}")
                nc.vector.tensor_scalar_mul(tmp[:], Grows_sb[:, rh, :],
                                            dvals_sb[:, rh : rh + 1])
                nc.vector.tensor_add(tmp[:], tmp[:], retps[rh][:, :128])
                nc.vector.tensor_add(tmp[:], tmp[:], biasb_sb[:])
                nc.sync.dma_start(out[rh * 128 : (rh + 1) * 128, :], tmp[:])

    nc.finalize()
    return nc


def kernel(H_v, H_e, adj_e, adj_v, T, weight, p, bias):
    global last_exec_time_ns, last_results
    _ensure_ntff_hook()
    from concourse.bass_utils import run_bass_kernel_spmd

    H_v = np.asarray(H_v, np.float32)
    H_e = np.asarray(H_e, np.float32)
    adj_v = np.asarray(adj_v, np.float32)
    T = np.asarray(T, np.float32)
    weight = np.asarray(weight, np.float32)
    p = np.asarray(p, np.float32).reshape(1, IN_E)
    bias = np.asarray(bias, np.float32).reshape(OUT_V)

    if "nc" not in _cache:
        _cache["nc"] = _build()
    nc = _cache["nc"]

    # ---- host-side sharding / layout prep ----
    TT = np.ascontiguousarray(T.T)                                   # [NE, NV]
    TTb = TT.astype(BF16)
    TTr = TTb.reshape(KO, 128, NV)
    HeT = np.ascontiguousarray(
        H_e.astype(BF16).reshape(KO, 128, IN_E).transpose(2, 0, 1)) # [128, KO, 128]
    pT = np.ascontiguousarray(p.T.astype(BF16))                      # [128, 1]
    A0 = adj_v.copy()
    np.fill_diagonal(A0, 0.0)
    diag = np.ascontiguousarray(np.diag(adj_v))                      # [NV]
    HvT = np.ascontiguousarray(H_v.T.astype(BF16))                   # [128, NV]
    HvT3 = HvT.reshape(IN_V, JT, 128)
    Wb = weight.astype(BF16)
    bias_b = np.ascontiguousarray(
        np.broadcast_to(bias.reshape(1, OUT_V), (128, OUT_V)))
    eye_b = np.eye(128, dtype=BF16)

    in_maps = []
    for c in range(NCORES):
        rows = slice(c * R, (c + 1) * R)
        slab_c = np.ascontiguousarray(
            TTb[:, rows].reshape(KO, 128, R).transpose(1, 0, 2))     # [128, KO, R]
        adjR_c = np.ascontiguousarray(
            A0[rows, :].reshape(2, 128, NV).transpose(1, 0, 2))      # [128, 2, NV]
        dvals_c = np.ascontiguousarray(diag[rows].reshape(2, 128).T) # [128, 2]
        HvTr_c = np.ascontiguousarray(HvT[:, rows])                  # [128, R]
        in_maps.append({
            "TTr": TTr, "slab": slab_c, "HeT": HeT, "pT": pT,
            "adjR": adjR_c, "HvT": HvT3, "HvTr": HvTr_c, "W": Wb,
            "dvals": dvals_c, "biasb": bias_b, "eye": eye_b,
        })

    trace = bool(int(os.environ.get("KERNEL_TRACE", "0")))
    res = run_bass_kernel_spmd(nc, in_maps, list(range(NCORES)), trace=trace)
    last_exec_time_ns = res.exec_time_ns
    last_results = res

    ret = np.concatenate([res.results[c]["out"] for c in range(NCORES)], axis=0)
    return (ret, H_e)
